# revision 60
# baseline (speedup 1.0000x reference)
"""Multi-head causal attention (B=2, T=4096, D=1024, H=16) on 8 trn2 cores.

Sharding: core c = 4*b + g handles batch b and head-group g (4 heads).
Fully fused single-pass kernel: for each q-block i (256 tokens) the causal
SDPA k-loop runs with exp on ACT as the critical resource; QKV projections
for upcoming blocks, the output projection of the previous block, and DMA
all ride in the PE/DMA slack of the same loop.

Key structure per k-block j (128 keys):
  S^T: 4 matmuls (one per head, 64-dim contraction) -> stp PSUM [128,1024]
  exp: one ACT activation [128,1024] -> et bf16 SBUF
  PV:  et 128x128 chunks as *stationary*, V (64 cols) + ones (1 col) as
       moving -> ctx accumulates as [q,d] in PSUM, softmax denominator Z
       accumulates as a PSUM column -> per-partition normalization.
  ctx.T via PE transpose (bf16, identity matmul) feeds the output
  projection; projections/out-proj ride as PE fillers in the SDPA loop.

Host sums the 4 per-core partial output projections per batch and adds bo.
"""
import numpy as np

B, T0, D, H = 2, 4096, 1024, 16
DK = D // H          # 64
NCORES = 8
HPC = H // 4         # 4 heads per core
CW = HPC * DK        # 256 head-columns per core

_BUILD_CACHE = {}

VG = 260             # vt column group per k-block: 4*64 V + 1 ones + 3 pad


def _build(T, variant=""):
    import concourse.bacc as bacc
    import concourse.mybir as mybir
    import concourse.tile as tile
    from contextlib import ExitStack

    F32 = mybir.dt.float32
    F32R = mybir.dt.float32r
    BF16 = mybir.dt.bfloat16
    EXP = mybir.ActivationFunctionType.Exp
    GE = mybir.AluOpType.is_ge

    NI = T // 256    # q-blocks of 256
    NJ = T // 128    # k-blocks of 128

    nc = bacc.Bacc("TRN2", target_bir_lowering=False, debug=False, num_devices=8)

    xt_d = nc.dram_tensor("xt", [D, T], F32R, kind="ExternalInput")
    wq_d = nc.dram_tensor("wq", [128, 8 * 256], F32R, kind="ExternalInput")
    wk_d = nc.dram_tensor("wk", [128, 8 * 256], F32R, kind="ExternalInput")
    wv_d = nc.dram_tensor("wv", [128, 8 * 256], F32R, kind="ExternalInput")
    wo_d = nc.dram_tensor("wo", [128, 2 * 1024], BF16, kind="ExternalInput")
    bqc_d = nc.dram_tensor("bqc", [128, 2], F32, kind="ExternalInput")
    bkc_d = nc.dram_tensor("bkc", [128, 2], F32, kind="ExternalInput")
    bvb_d = nc.dram_tensor("bvb", [128, 256], F32, kind="ExternalInput")
    idn_d = nc.dram_tensor("ident", [128, 128], BF16, kind="ExternalInput")
    zer_d = nc.dram_tensor("zer64", [64, T], F32R, kind="ExternalInput")
    ot_d = nc.dram_tensor("ot", [D, T], F32, kind="ExternalOutput")
    dbg = "dbg" in variant
    if dbg or "dA" in variant or "dB" in variant:
        NJ_ = T // 128
        qt_o = nc.dram_tensor("qt_o", [2 * 128, T], F32R, kind="ExternalOutput")
        kt_o = nc.dram_tensor("kt_o", [2 * 128, T], F32R, kind="ExternalOutput")
        vt_o = nc.dram_tensor("vt_o", [128, NJ_ * VG], BF16, kind="ExternalOutput")
        ua_o = nc.dram_tensor("ua_o", [T // 256 * 128, 512], F32, kind="ExternalOutput")
        zz_o = nc.dram_tensor("zz_o", [T // 256 * 128, 8], F32, kind="ExternalOutput")
        cx_o = nc.dram_tensor("cx_o", [T // 256 * 128, 512], BF16, kind="ExternalOutput")

    with tile.TileContext(nc) as tc, ExitStack() as ctx:
        ctx.enter_context(nc.allow_low_precision(reason="fp32r/bf16 by design"))

        # ---- persistent SBUF ----
        # kta[p]: rows 0:64 = K^T of head 2p, rows 64:128 zero.
        # ktb[p]: rows 64:128 = K^T of head 2p+1, rows 0:64 zero.
        # (Full-128-partition S matmuls; 64-partition operands crash HW.)
        per = ctx.enter_context(tc.tile_pool(name="persist", bufs=1))
        kta = [per.tile([128, T], F32R, name=f"kta{p}", tag=f"kta{p}")
               for p in range(2)]
        ktb = [per.tile([128, T], F32R, name=f"ktb{p}", tag=f"ktb{p}")
               for p in range(2)]
        vt = per.tile([128, NJ * VG], BF16, name="vt", tag="vt")
        wq_sb = per.tile([128, 2048], F32R, name="wq", tag="wq")
        wk_sb = per.tile([128, 2048], F32R, name="wk", tag="wk")
        wv_sb = per.tile([128, 2048], F32R, name="wv", tag="wv")
        wo_sb = per.tile([128, 2, 1024], BF16, name="wo", tag="wo")
        bqc_sb = per.tile([128, 2], F32, name="bqc", tag="bqc")
        bkc_sb = per.tile([128, 2], F32, name="bkc", tag="bkc")
        bvb_sb = per.tile([128, 256], F32, name="bvb", tag="bvb")
        idn_sb = per.tile([128, 128], BF16, name="idn", tag="idn")

        # ones column per k-group (for Z matmuls); pad cols stay whatever.
        vt4 = vt[:].rearrange("p (j c) -> p j c", c=VG)[:, :, 256:VG]
        nc.vector.memset(vt4, 1.0)

        # ---- pools ----
        xpool = ctx.enter_context(tc.tile_pool(name="xts", bufs=16))
        etp = ctx.enter_context(tc.tile_pool(name="etp", bufs=9))
        qtp = ctx.enter_context(tc.tile_pool(name="qtp", bufs=3))
        cxp = ctx.enter_context(tc.tile_pool(name="cxp", bufs=2))
        cxtp = ctx.enter_context(tc.tile_pool(name="cxtp", bufs=20))
        otp = ctx.enter_context(tc.tile_pool(name="otp", bufs=1))
        uap = ctx.enter_context(tc.tile_pool(name="uap", bufs=2))

        stp_pool = ctx.enter_context(tc.tile_pool(name="stps", bufs=2, space="PSUM"))
        ua_pool = ctx.enter_context(tc.tile_pool(name="uaps", bufs=1, space="PSUM"))
        pp_pool = ctx.enter_context(tc.tile_pool(name="ppps", bufs=2, space="PSUM"))
        po_pool = pp_pool  # shared ring: proj and output-proj tiles rotate together

        xts = {}          # gen -> list of 8 tiles [128, 512]

        def dma_x(gen, half=None):
            if gen * 512 >= T:
                return
            if gen not in xts:
                xts[gen] = [xpool.tile([128, 512], F32R, name="xt", tag="xt")
                            for _ in range(8)]
            lo, hi = {None: (0, 512), 0: (0, 256), 1: (256, 512)}[half]
            for db, xtile in enumerate(xts[gen]):
                nc.sync.dma_start(
                    xtile[:, lo:hi],
                    xt_d.ap()[db * 128:(db + 1) * 128,
                              gen * 512 + lo:gen * 512 + hi],
                )

        qts = {}          # (i, p) -> (q^T tile [128, 512], col offset)

        def proj_qk(bp, p, which):
            """Project q or k for block-pair bp (blocks 2bp, 2bp+1), pair p."""
            if 2 * bp >= NI:
                return
            w_sb, bias = (wq_sb, bqc_sb) if which == "q" else (wk_sb, bkc_sb)
            ps = pp_pool.tile([128, 512], F32, name="pp", tag="pp")
            for db in range(8):
                nc.tensor.matmul(
                    ps[:],
                    w_sb[:, db * 256 + p * 128: db * 256 + (p + 1) * 128],
                    xts[bp][db][:],
                    start=(db == 0), stop=(db == 7),
                )
            if which == "q":
                qtile = qtp.tile([128, 512], F32R, name="qt", tag=f"qt{p}")
                nc.vector.tensor_scalar_add(qtile[:], ps[:], bias[:, p:p + 1])
                qts[(2 * bp, p)] = (qtile, 0)
                qts[(2 * bp + 1, p)] = (qtile, 256)
            else:
                nc.vector.tensor_scalar_add(
                    kta[p][0:64, bp * 512:(bp + 1) * 512], ps[0:64, :],
                    bias[0:64, p:p + 1]
                )
                nc.vector.tensor_scalar_add(
                    ktb[p][64:128, bp * 512:(bp + 1) * 512], ps[64:128, :],
                    bias[64:128, p:p + 1]
                )

        def proj_v(i, jb):
            """Project v for k-block j = 2*i + jb -> vt group."""
            if i >= NI:
                return
            gen, off = divmod(i, 2)
            j = 2 * i + jb
            ps = pp_pool.tile([128, 256], F32, name="pp", tag="pp")
            tok = off * 256 + jb * 128
            for db in range(8):
                nc.tensor.matmul(
                    ps[:],
                    xts[gen][db][:, tok:tok + 128],
                    wv_sb[:, db * 256:(db + 1) * 256],
                    start=(db == 0), stop=(db == 7),
                )
            nc.vector.tensor_add(
                vt[:, j * VG: j * VG + 256], ps[:], bvb_sb[:]
            )

        ot_stage = {}     # ob -> staging tile [128, 512] across an i-pair

        def po_unit(i, ob, cxT):
            """Output projection for q-block i, output row block ob."""
            po = po_pool.tile([128, 256], F32, name="po", tag="pp")
            for qc in range(2):
                for h in range(2):
                    if "nopo" in variant:
                        continue
                    nc.tensor.matmul(
                        po[:, qc * 128:(qc + 1) * 128],
                        wo_sb[:, h, ob * 128:(ob + 1) * 128],
                        cxT[qc][:, h, :],
                        start=(h == 0), stop=(h == 1),
                    )
            if "nopo" in variant:
                nc.vector.memset(po[:], 0.0)
            last_pair = (i // 2 == NI // 2 - 1)
            if i % 2 == 0:
                stage = otp.tile([128, 512], F32, name="ots", tag=f"ots{ob}")
                ot_stage[ob] = stage
                nc.vector.tensor_copy(stage[:, 0:256], po[:])
                if last_pair:
                    nc.sync.dma_start(
                        ot_d.ap()[ob * 128:(ob + 1) * 128,
                                  i * 256:(i + 1) * 256],
                        stage[:, 0:256],
                    )
            else:
                stage = ot_stage[ob]
                nc.vector.tensor_copy(stage[:, 256:512], po[:])
                if last_pair:
                    nc.sync.dma_start(
                        ot_d.ap()[ob * 128:(ob + 1) * 128,
                                  i * 256:(i + 1) * 256],
                        stage[:, 256:512],
                    )
                else:
                    nc.sync.dma_start(
                        ot_d.ap()[ob * 128:(ob + 1) * 128,
                                  (i - 1) * 256:(i + 1) * 256],
                        stage[:],
                    )

        po_sets = []      # deferred output-projection work: (block, cxT)
        CH = 4            # k-blocks per PSUM accumulation chunk

        def norm_unit(i, uacc):
            """Normalize block i's ctx, transpose it, queue its out-proj."""
            if dbg or "dA" in variant:
                nc.sync.dma_start(ua_o.ap()[i * 128:(i + 1) * 128, :],
                                  uacc[:, 0:512])
                nc.sync.dma_start(zz_o.ap()[i * 128:(i + 1) * 128, :],
                                  uacc[:, 512:520])
            rz = cxp.tile([128, 8], F32, name="rz", tag="rz")
            nc.vector.reciprocal_approx_fast(out=rz[:], in_=uacc[:, 512:520])
            cxT = []
            for qc in range(2):
                cx = cxp.tile([128, 256], BF16, name="cx", tag="cx")
                for h in range(4):
                    nc.vector.tensor_scalar_mul(
                        cx[:, 64 * h: 64 * (h + 1)],
                        uacc[:, 256 * qc + 64 * h: 256 * qc + 64 * (h + 1)],
                        rz[:, 4 * qc + h: 4 * qc + h + 1],
                    )
                ct = cxtp.tile([128, 2, 128], BF16, name="cxT", tag="cxT")
                if "xbar" in variant:
                    nc.sync.dma_start_transpose(ct[:], cx[:])
                else:
                    tps = po_pool.tile([128, 256], BF16, name="tp", tag="pp")
                    for hh in range(2):
                        nc.tensor.transpose(
                            tps[:, 128 * hh:128 * (hh + 1)],
                            cx[:, 128 * hh:128 * (hh + 1)],
                            idn_sb[:],
                        )
                    nc.vector.tensor_copy(
                        ct[:].rearrange("p a b -> p (a b)"), tps[:]
                    )
                if dbg or "dB" in variant:
                    nc.sync.dma_start(
                        cx_o.ap()[i * 128:(i + 1) * 128,
                                  qc * 256:(qc + 1) * 256], cx[:]
                    )
                cxT.append(ct)
            po_sets.append((i, cxT))

        def pv_units_for(i, cidx, ets, ua, uacc):
            """Closed PV+Z accumulation group closures for one chunk + drain.

            PV groups live in bank 0 of `ua` (cols 0:512), Z groups in
            bank 1 (cols 512:520); groups are sequential per bank so each
            start/stop pair owns its zero-region exclusively.
            """
            def group(qc, h):
                if "nopv" in variant:
                    return
                js = [(et, j) for (et, j) in ets
                      if not (j == 2 * i + 1 and qc == 0)]
                for idx, (et, j) in enumerate(js):
                    flags = dict(
                        start=(idx == 0), stop=(idx == len(js) - 1),
                        skip_group_check=True,
                    )
                    lhs = et[:, 256 * h + 128 * qc: 256 * h + 128 * (qc + 1)]
                    nc.tensor.matmul(
                        ua[:, 256 * qc + 64 * h: 256 * qc + 64 * (h + 1)],
                        lhs,
                        vt[:, j * VG + 64 * h: j * VG + 64 * (h + 1)],
                        **flags,
                    )
                    if "nozc" not in variant:
                        nc.tensor.matmul(
                            ua[:, 512 + 4 * qc + h: 512 + 4 * qc + h + 1],
                            lhs,
                            vt[:, j * VG + 256: j * VG + 257],
                            **flags,
                        )
                    elif h == 0:
                        nc.tensor.matmul(
                            ua[:, 512 + 4 * qc: 512 + 4 * qc + 4],
                            lhs,
                            vt[:, j * VG + 64: j * VG + 68],
                            **flags,
                        )

            def drain():
                if "nopv" in variant:
                    if cidx == 0:
                        nc.vector.memset(uacc[:], 1.0)
                    return
                if cidx == 0:
                    nc.vector.tensor_copy(uacc[:], ua[:])
                else:
                    nc.vector.tensor_add(uacc[:], uacc[:], ua[:])

            out = [lambda qc=qc, h=h: group(qc, h)
                   for qc in range(2) for h in range(4)]
            out.append(drain)
            return out

        # ---- bootstrap (DMA order matters: x halves + qk weights first) ----
        dma_x(0, half=0)
        nc.sync.dma_start(wq_sb[:], wq_d.ap()[:])
        nc.sync.dma_start(bqc_sb[:], bqc_d.ap()[:])
        dma_x(0, half=1)
        nc.sync.dma_start(wk_sb[:], wk_d.ap()[:])
        nc.sync.dma_start(bkc_sb[:], bkc_d.ap()[:])
        for p in range(2):
            nc.sync.dma_start(kta[p][64:128, :], zer_d.ap()[:])
            nc.sync.dma_start(ktb[p][0:64, :], zer_d.ap()[:])
        for p in range(2):
            proj_qk(0, p, "q")
        nc.sync.dma_start(wv_sb[:], wv_d.ap()[:])
        nc.sync.dma_start(bvb_sb[:], bvb_d.ap()[:])
        for p in range(2):
            proj_qk(0, p, "k")
        nc.sync.dma_start(wo_sb[:].rearrange("p a b -> p (a b)"), wo_d.ap()[:])
        nc.sync.dma_start(idn_sb[:], idn_d.ap()[:])
        dma_x(1)

        pv_units = []   # pending PV/drain/norm closures, carried across blocks
        for i in range(NI):
            jmax = 2 * i + 2
            uacc = uap.tile([128, 520], F32, name="uacc", tag="uacc")

            # filler units to interleave into this i's j-loop
            units = []
            if i == 0:
                units.append(lambda: proj_v(0, 0))
                units.append(lambda: proj_v(0, 1))
            if i % 2 == 0:
                units.append(lambda g=i // 2 + 2: dma_x(g))
                for p in range(2):
                    units.append(lambda p=p, b=i // 2 + 1: proj_qk(b, p, "q"))
                for p in range(2):
                    units.append(lambda p=p, b=i // 2 + 1: proj_qk(b, p, "k"))
            units.append(lambda i=i: proj_v(i + 1, 0))
            units.append(lambda i=i: proj_v(i + 1, 1))
            def absorb_one():
                if po_sets:
                    b, c = po_sets.pop(0)
                    for ob in range(8):
                        po_unit(b, ob, c)
            if i >= NI // 2:
                for _ in range(3):
                    units.append(absorb_one)
            if "nofill" in variant:
                for u in units:
                    u()
                units = []

            chunks = [list(range(c0, min(c0 + CH, jmax)))
                      for c0 in range(0, jmax, CH)]
            for cidx, chunk in enumerate(chunks):
                ets = []
                for jn, j in enumerate(chunk):
                    t = j - 2 * i   # 0/1 on the diagonal, <0 off-diagonal
                    stp = stp_pool.tile([128, 1024], F32, name="stp", tag="stp")
                    for p in range(2):
                        for half in range(2):
                            h = 2 * p + half
                            ktx = kta[p] if half == 0 else ktb[p]
                            qtile, qoff = qts[(i, p)]
                            nc.tensor.matmul(
                                stp[:, 256 * h: 256 * (h + 1)],
                                ktx[:, j * 128:(j + 1) * 128],
                                qtile[:, qoff:qoff + 256],
                                start=True, stop=True,
                            )
                    et = etp.tile([128, 1024], BF16, name="et", tag="et")
                    nc.scalar.activation(et[:], stp[:], EXP, scale=0.125)
                    if t >= 0 and "nosel" not in variant:
                        nc.gpsimd.affine_select(
                            out=et[:].rearrange("p (h w) -> p h w", h=4),
                            in_=et[:].rearrange("p (h w) -> p h w", h=4),
                            compare_op=GE, fill=0.0,
                            base=-128 * t, pattern=[[0, 4], [1, 256]],
                            channel_multiplier=-1,
                        )
                    ets.append((et, j))

                    # interleave fillers + previous chunk's PV groups
                    n_emit = -(-len(units) // (jmax - j))
                    for _ in range(n_emit):
                        units.pop(0)()
                    slots_left = len(chunk) - jn
                    n_pv = -(-len(pv_units) // slots_left)
                    for _ in range(n_pv):
                        pv_units.pop(0)()
                ua = (None if "nopv" in variant else
                      ua_pool.tile([128, 520], F32, name="ua", tag="ua"))
                pv_units += pv_units_for(i, cidx, ets, ua, uacc)
                if cidx == len(chunks) - 1:
                    pv_units.append(lambda i=i, u=uacc: norm_unit(i, u))
            for u in units:
                u()

        # tail: drain pending PV/norm + deferred output projections
        for u in pv_units:
            u()
        for b, c in po_sets:
            for ob in range(8):
                po_unit(b, ob, c)
        if dbg:
            for p in range(2):
                nc.sync.dma_start(kt_o.ap()[p * 128:(p + 1) * 128, :], kta[p][:])
            nc.sync.dma_start(vt_o.ap()[:], vt[:])

    nc.compile()
    return nc


def _get_built(T, variant=""):
    key = (T, variant)
    if key not in _BUILD_CACHE:
        _BUILD_CACHE[key] = _build(T, variant)
    return _BUILD_CACHE[key]


def _rearr_w(w):  # [1024, 256] -> [128, 8*256] (d-block major free dim)
    return np.ascontiguousarray(
        w.reshape(8, 128, 256).transpose(1, 0, 2).reshape(128, 8 * 256)
    )


def _numpy_ref(x, mask, Wq, bq, Wk, bk, Wv, bv, Wo, bo):
    T = x.shape[1]
    q = (x @ Wq + bq).reshape(B, T, H, DK).transpose(0, 2, 1, 3)
    k = (x @ Wk + bk).reshape(B, T, H, DK).transpose(0, 2, 1, 3)
    v = (x @ Wv + bv).reshape(B, T, H, DK).transpose(0, 2, 1, 3)
    s = np.einsum("bhqd,bhkd->bhqk", q, k) / np.sqrt(np.float32(DK))
    s = np.where(mask, s, s - 1e9)
    s = s - s.max(axis=-1, keepdims=True)
    e = np.exp(s)
    p = e / e.sum(axis=-1, keepdims=True)
    o = np.einsum("bhqk,bhkd->bhqd", p, v).transpose(0, 2, 1, 3).reshape(B, T, D)
    return (o @ Wo + bo).astype(np.float32)


def kernel(x, mask, Wq, bq, Wk, bk, Wv, bv, Wo, bo):
    from concourse import bass_utils

    x = np.ascontiguousarray(np.asarray(x, dtype=np.float32))
    mask = np.asarray(mask)
    T = x.shape[1]

    causal = bool(
        np.array_equal(mask[0, 0], np.tril(np.ones((T, T), dtype=bool)))
    )
    if not causal or x.shape != (B, T, D) or T % 512 != 0:
        return _numpy_ref(
            np.asarray(x, np.float32), mask,
            np.asarray(Wq, np.float32), np.asarray(bq, np.float32),
            np.asarray(Wk, np.float32), np.asarray(bk, np.float32),
            np.asarray(Wv, np.float32), np.asarray(bv, np.float32),
            np.asarray(Wo, np.float32), np.asarray(bo, np.float32),
        )

    in_maps = _make_in_maps(dict(x=x, Wq=Wq, bq=bq, Wk=Wk, bk=bk,
                                 Wv=Wv, bv=bv, Wo=Wo))
    nc = _get_built(T)
    res = bass_utils.run_bass_kernel_spmd(nc, in_maps, core_ids=list(range(NCORES)))

    out = np.zeros((B, T, D), np.float32)
    for c in range(NCORES):
        out[c // 4] += res.results[c]["ot"].T
    out += np.asarray(bo, np.float32)
    return out


def _make_in_maps(inputs):
    import ml_dtypes
    x = np.ascontiguousarray(np.asarray(inputs["x"], np.float32))
    T = x.shape[1]
    Wq = np.asarray(inputs["Wq"], np.float32)
    Wk = np.asarray(inputs["Wk"], np.float32)
    Wv = np.asarray(inputs["Wv"], np.float32)
    Wo = np.asarray(inputs["Wo"], np.float32)
    bq = np.asarray(inputs["bq"], np.float32)
    bk = np.asarray(inputs["bk"], np.float32)
    bv = np.asarray(inputs["bv"], np.float32)

    xts = [np.ascontiguousarray(x[b].T) for b in range(B)]

    in_maps = []
    for c in range(NCORES):
        b, g = divmod(c, 4)
        cols = slice(g * CW, (g + 1) * CW)
        rows = slice(g * CW, (g + 1) * CW)
        wo_g = Wo[rows].astype(ml_dtypes.bfloat16)  # [256, 1024]
        in_maps.append({
            "xt": xts[b],
            "wq": _rearr_w(Wq[:, cols]),
            "wk": _rearr_w(Wk[:, cols]),
            "wv": _rearr_w(Wv[:, cols]),
            "wo": np.ascontiguousarray(
                wo_g.reshape(2, 128, 1024).transpose(1, 0, 2).reshape(128, 2048)
            ),
            "bqc": np.ascontiguousarray(bq[cols].reshape(2, 128).T),
            "bkc": np.ascontiguousarray(bk[cols].reshape(2, 128).T),
            "bvb": np.ascontiguousarray(
                np.broadcast_to(bv[cols][None, :], (128, 256)).copy()
            ),
            "ident": np.eye(128, dtype=ml_dtypes.bfloat16),
            "zer64": np.zeros((64, T), np.float32),
        })

    return in_maps


# revision 61
# speedup vs baseline: 1.0316x; 1.0316x over previous
"""Multi-head causal attention (B=2, T=4096, D=1024, H=16) on 8 trn2 cores.

Sharding: core c = 4*b + g handles batch b and head-group g (4 heads).
Fully fused single-pass kernel: for each q-block i (256 tokens) the causal
SDPA k-loop runs with exp on ACT as the critical resource; QKV projections
for upcoming blocks, the output projection of the previous block, and DMA
all ride in the PE/DMA slack of the same loop.

Key structure per k-block j (128 keys):
  S^T: 4 matmuls (one per head, 64-dim contraction) -> stp PSUM [128,1024]
  exp: one ACT activation [128,1024] -> et bf16 SBUF
  PV:  et 128x128 chunks as *stationary*, V (64 cols) + ones (1 col) as
       moving -> ctx accumulates as [q,d] in PSUM, softmax denominator Z
       accumulates as a PSUM column -> per-partition normalization.
  ctx.T via PE transpose (bf16, identity matmul) feeds the output
  projection; projections/out-proj ride as PE fillers in the SDPA loop.

Host sums the 4 per-core partial output projections per batch and adds bo.
"""
import numpy as np

B, T0, D, H = 2, 4096, 1024, 16
DK = D // H          # 64
NCORES = 8
HPC = H // 4         # 4 heads per core
CW = HPC * DK        # 256 head-columns per core

_BUILD_CACHE = {}

VG = 260             # vt column group per k-block: 4*64 V + 1 ones + 3 pad


def _build(T, variant=""):
    import concourse.bacc as bacc
    import concourse.mybir as mybir
    import concourse.tile as tile
    from contextlib import ExitStack

    F32 = mybir.dt.float32
    F32R = mybir.dt.float32r
    BF16 = mybir.dt.bfloat16
    EXP = mybir.ActivationFunctionType.Exp
    GE = mybir.AluOpType.is_ge

    NI = T // 256    # q-blocks of 256
    NJ = T // 128    # k-blocks of 128

    nc = bacc.Bacc("TRN2", target_bir_lowering=False, debug=False, num_devices=8)

    xt_d = nc.dram_tensor("xt", [D, T], F32R, kind="ExternalInput")
    wq_d = nc.dram_tensor("wq", [128, 8 * 256], F32R, kind="ExternalInput")
    wk_d = nc.dram_tensor("wk", [128, 8 * 256], F32R, kind="ExternalInput")
    wv_d = nc.dram_tensor("wv", [128, 8 * 256], F32R, kind="ExternalInput")
    wo_d = nc.dram_tensor("wo", [128, 2 * 1024], BF16, kind="ExternalInput")
    bqc_d = nc.dram_tensor("bqc", [128, 2], F32, kind="ExternalInput")
    bkc_d = nc.dram_tensor("bkc", [128, 2], F32, kind="ExternalInput")
    bvb_d = nc.dram_tensor("bvb", [128, 256], F32, kind="ExternalInput")
    idn_d = nc.dram_tensor("ident", [128, 128], BF16, kind="ExternalInput")
    zer_d = nc.dram_tensor("zer64", [64, T], F32R, kind="ExternalInput")
    ot_d = nc.dram_tensor("ot", [D, T], F32, kind="ExternalOutput")
    dbg = "dbg" in variant
    if dbg or "dA" in variant or "dB" in variant:
        NJ_ = T // 128
        qt_o = nc.dram_tensor("qt_o", [2 * 128, T], F32R, kind="ExternalOutput")
        kt_o = nc.dram_tensor("kt_o", [2 * 128, T], F32R, kind="ExternalOutput")
        vt_o = nc.dram_tensor("vt_o", [128, NJ_ * VG], BF16, kind="ExternalOutput")
        ua_o = nc.dram_tensor("ua_o", [T // 256 * 128, 512], F32, kind="ExternalOutput")
        zz_o = nc.dram_tensor("zz_o", [T // 256 * 128, 8], F32, kind="ExternalOutput")
        cx_o = nc.dram_tensor("cx_o", [T // 256 * 128, 512], BF16, kind="ExternalOutput")

    with tile.TileContext(nc) as tc, ExitStack() as ctx:
        ctx.enter_context(nc.allow_low_precision(reason="fp32r/bf16 by design"))

        # ---- persistent SBUF ----
        # kta[p]: rows 0:64 = K^T of head 2p, rows 64:128 zero.
        # ktb[p]: rows 64:128 = K^T of head 2p+1, rows 0:64 zero.
        # (Full-128-partition S matmuls; 64-partition operands crash HW.)
        per = ctx.enter_context(tc.tile_pool(name="persist", bufs=1))
        kta = [per.tile([128, T], F32R, name=f"kta{p}", tag=f"kta{p}")
               for p in range(2)]
        ktb = [per.tile([128, T], F32R, name=f"ktb{p}", tag=f"ktb{p}")
               for p in range(2)]
        vt = per.tile([128, NJ * VG], BF16, name="vt", tag="vt")
        wq_sb = per.tile([128, 2048], F32R, name="wq", tag="wq")
        wk_sb = per.tile([128, 2048], F32R, name="wk", tag="wk")
        wv_sb = per.tile([128, 2048], F32R, name="wv", tag="wv")
        wo_sb = per.tile([128, 2, 1024], BF16, name="wo", tag="wo")
        bqc_sb = per.tile([128, 2], F32, name="bqc", tag="bqc")
        bkc_sb = per.tile([128, 2], F32, name="bkc", tag="bkc")
        bvb_sb = per.tile([128, 256], F32, name="bvb", tag="bvb")
        idn_sb = per.tile([128, 128], BF16, name="idn", tag="idn")

        # ones column per k-group (for Z matmuls); pad cols stay whatever.
        vt4 = vt[:].rearrange("p (j c) -> p j c", c=VG)[:, :, 256:VG]
        nc.vector.memset(vt4, 1.0)

        # ---- pools ----
        xpool = ctx.enter_context(tc.tile_pool(name="xts", bufs=16))
        etp = ctx.enter_context(tc.tile_pool(name="etp", bufs=9))
        qtp = ctx.enter_context(tc.tile_pool(name="qtp", bufs=3))
        cxp = ctx.enter_context(tc.tile_pool(name="cxp", bufs=2))
        cxtp = ctx.enter_context(tc.tile_pool(name="cxtp", bufs=20))
        otp = ctx.enter_context(tc.tile_pool(name="otp", bufs=1))
        uap = ctx.enter_context(tc.tile_pool(name="uap", bufs=2))

        stp_pool = ctx.enter_context(tc.tile_pool(name="stps", bufs=2, space="PSUM"))
        ua_pool = ctx.enter_context(tc.tile_pool(name="uaps", bufs=1, space="PSUM"))
        pp_pool = ctx.enter_context(tc.tile_pool(name="ppps", bufs=2, space="PSUM"))
        po_pool = pp_pool  # shared ring: proj and output-proj tiles rotate together

        xts = {}          # gen -> list of 8 tiles [128, 512]

        def dma_x(gen, half=None):
            if gen * 512 >= T:
                return
            if gen not in xts:
                xts[gen] = [xpool.tile([128, 512], F32R, name="xt", tag="xt")
                            for _ in range(8)]
            lo, hi = {None: (0, 512), 0: (0, 256), 1: (256, 512)}[half]
            for db, xtile in enumerate(xts[gen]):
                nc.sync.dma_start(
                    xtile[:, lo:hi],
                    xt_d.ap()[db * 128:(db + 1) * 128,
                              gen * 512 + lo:gen * 512 + hi],
                )

        qts = {}          # (i, p) -> (q^T tile [128, 512], col offset)

        def proj_qk(bp, p, which):
            """Project q or k for block-pair bp (blocks 2bp, 2bp+1), pair p."""
            if 2 * bp >= NI:
                return
            w_sb, bias = (wq_sb, bqc_sb) if which == "q" else (wk_sb, bkc_sb)
            ps = pp_pool.tile([128, 512], F32, name="pp", tag="pp")
            for db in range(8):
                nc.tensor.matmul(
                    ps[:],
                    w_sb[:, db * 256 + p * 128: db * 256 + (p + 1) * 128],
                    xts[bp][db][:],
                    start=(db == 0), stop=(db == 7),
                )
            if which == "q":
                qtile = qtp.tile([128, 512], F32R, name="qt", tag=f"qt{p}")
                nc.vector.tensor_scalar_add(qtile[:], ps[:], bias[:, p:p + 1])
                qts[(2 * bp, p)] = (qtile, 0)
                qts[(2 * bp + 1, p)] = (qtile, 256)
            else:
                nc.vector.tensor_scalar_add(
                    kta[p][0:64, bp * 512:(bp + 1) * 512], ps[0:64, :],
                    bias[0:64, p:p + 1]
                )
                nc.vector.tensor_scalar_add(
                    ktb[p][64:128, bp * 512:(bp + 1) * 512], ps[64:128, :],
                    bias[64:128, p:p + 1]
                )

        def proj_v(i, jb):
            """Project v for k-block j = 2*i + jb -> vt group."""
            if i >= NI:
                return
            gen, off = divmod(i, 2)
            j = 2 * i + jb
            ps = pp_pool.tile([128, 256], F32, name="pp", tag="pp")
            tok = off * 256 + jb * 128
            for db in range(8):
                nc.tensor.matmul(
                    ps[:],
                    xts[gen][db][:, tok:tok + 128],
                    wv_sb[:, db * 256:(db + 1) * 256],
                    start=(db == 0), stop=(db == 7),
                )
            nc.vector.tensor_add(
                vt[:, j * VG: j * VG + 256], ps[:], bvb_sb[:]
            )

        ot_stage = {}     # ob -> staging tile [128, 512] across an i-pair

        def po_unit(i, ob, cxT):
            """Output projection for q-block i, output row block ob."""
            po = po_pool.tile([128, 256], F32, name="po", tag="pp")
            for qc in range(2):
                for h in range(2):
                    if "nopo" in variant:
                        continue
                    nc.tensor.matmul(
                        po[:, qc * 128:(qc + 1) * 128],
                        wo_sb[:, h, ob * 128:(ob + 1) * 128],
                        cxT[qc][:, h, :],
                        start=(h == 0), stop=(h == 1),
                    )
            if "nopo" in variant:
                nc.vector.memset(po[:], 0.0)
            last_pair = (i // 2 == NI // 2 - 1)
            if i % 2 == 0:
                stage = otp.tile([128, 512], F32, name="ots", tag=f"ots{ob}")
                ot_stage[ob] = stage
                nc.vector.tensor_copy(stage[:, 0:256], po[:])
                if last_pair:
                    nc.sync.dma_start(
                        ot_d.ap()[ob * 128:(ob + 1) * 128,
                                  i * 256:(i + 1) * 256],
                        stage[:, 0:256],
                    )
            else:
                stage = ot_stage[ob]
                nc.vector.tensor_copy(stage[:, 256:512], po[:])
                if last_pair:
                    nc.sync.dma_start(
                        ot_d.ap()[ob * 128:(ob + 1) * 128,
                                  i * 256:(i + 1) * 256],
                        stage[:, 256:512],
                    )
                else:
                    nc.sync.dma_start(
                        ot_d.ap()[ob * 128:(ob + 1) * 128,
                                  (i - 1) * 256:(i + 1) * 256],
                        stage[:],
                    )

        po_sets = []      # deferred output-projection work: (block, cxT)
        CH = 4            # k-blocks per PSUM accumulation chunk

        def norm_unit(i, uacc):
            """Normalize block i's ctx, transpose it, queue its out-proj."""
            if dbg or "dA" in variant:
                nc.sync.dma_start(ua_o.ap()[i * 128:(i + 1) * 128, :],
                                  uacc[:, 0:512])
                nc.sync.dma_start(zz_o.ap()[i * 128:(i + 1) * 128, :],
                                  uacc[:, 512:520])
            rz = cxp.tile([128, 8], F32, name="rz", tag="rz")
            nc.vector.reciprocal_approx_fast(out=rz[:], in_=uacc[:, 512:520])
            cxT = []
            for qc in range(2):
                cx = cxp.tile([128, 256], BF16, name="cx", tag="cx")
                for h in range(4):
                    nc.vector.tensor_scalar_mul(
                        cx[:, 64 * h: 64 * (h + 1)],
                        uacc[:, 256 * qc + 64 * h: 256 * qc + 64 * (h + 1)],
                        rz[:, 4 * qc + h: 4 * qc + h + 1],
                    )
                ct = cxtp.tile([128, 2, 128], BF16, name="cxT", tag="cxT")
                if "xbar" in variant:
                    nc.sync.dma_start_transpose(ct[:], cx[:])
                else:
                    tps = po_pool.tile([128, 256], BF16, name="tp", tag="pp")
                    for hh in range(2):
                        nc.tensor.transpose(
                            tps[:, 128 * hh:128 * (hh + 1)],
                            cx[:, 128 * hh:128 * (hh + 1)],
                            idn_sb[:],
                        )
                    nc.vector.tensor_copy(
                        ct[:].rearrange("p a b -> p (a b)"), tps[:]
                    )
                if dbg or "dB" in variant:
                    nc.sync.dma_start(
                        cx_o.ap()[i * 128:(i + 1) * 128,
                                  qc * 256:(qc + 1) * 256], cx[:]
                    )
                cxT.append(ct)
            po_sets.append((i, cxT))

        def pv_units_for(i, cidx, ets, ua, uacc):
            """Closed PV+Z accumulation group closures for one chunk + drain.

            PV groups live in bank 0 of `ua` (cols 0:512), Z groups in
            bank 1 (cols 512:520); groups are sequential per bank so each
            start/stop pair owns its zero-region exclusively.
            """
            def group(qc, h):
                if "nopv" in variant:
                    return
                js = [(et, j) for (et, j) in ets
                      if not (j == 2 * i + 1 and qc == 0)]
                for idx, (et, j) in enumerate(js):
                    flags = dict(
                        start=(idx == 0), stop=(idx == len(js) - 1),
                        skip_group_check=True,
                    )
                    lhs = et[:, 256 * h + 128 * qc: 256 * h + 128 * (qc + 1)]
                    nc.tensor.matmul(
                        ua[:, 256 * qc + 64 * h: 256 * qc + 64 * (h + 1)],
                        lhs,
                        vt[:, j * VG + 64 * h: j * VG + 64 * (h + 1)],
                        **flags,
                    )
                    if "nozc" not in variant:
                        nc.tensor.matmul(
                            ua[:, 512 + 4 * qc + h: 512 + 4 * qc + h + 1],
                            lhs,
                            vt[:, j * VG + 256: j * VG + 257],
                            **flags,
                        )
                    elif h == 0:
                        nc.tensor.matmul(
                            ua[:, 512 + 4 * qc: 512 + 4 * qc + 4],
                            lhs,
                            vt[:, j * VG + 64: j * VG + 68],
                            **flags,
                        )

            def drain():
                if "nopv" in variant:
                    if cidx == 0:
                        nc.vector.memset(uacc[:], 1.0)
                    return
                if cidx == 0:
                    nc.vector.tensor_copy(uacc[:], ua[:])
                else:
                    nc.vector.tensor_add(uacc[:], uacc[:], ua[:])

            out = [lambda qc=qc, h=h: group(qc, h)
                   for qc in range(2) for h in range(4)]
            out.append(drain)
            return out

        # ---- bootstrap (DMA order matters: x halves + qk weights first) ----
        dma_x(0, half=0)
        nc.sync.dma_start(wq_sb[:], wq_d.ap()[:])
        nc.sync.dma_start(bqc_sb[:], bqc_d.ap()[:])
        dma_x(0, half=1)
        nc.sync.dma_start(wk_sb[:], wk_d.ap()[:])
        nc.sync.dma_start(bkc_sb[:], bkc_d.ap()[:])
        for p in range(2):
            nc.sync.dma_start(kta[p][64:128, :], zer_d.ap()[:])
            nc.sync.dma_start(ktb[p][0:64, :], zer_d.ap()[:])
        for p in range(2):
            proj_qk(0, p, "q")
        nc.sync.dma_start(wv_sb[:], wv_d.ap()[:])
        nc.sync.dma_start(bvb_sb[:], bvb_d.ap()[:])
        for p in range(2):
            proj_qk(0, p, "k")
        nc.sync.dma_start(wo_sb[:].rearrange("p a b -> p (a b)"), wo_d.ap()[:])
        nc.sync.dma_start(idn_sb[:], idn_d.ap()[:])
        dma_x(1)

        pv_units = []   # pending PV/drain/norm closures, carried across blocks
        for i in range(NI):
            jmax = 2 * i + 2
            uacc = uap.tile([128, 520], F32, name="uacc", tag="uacc")

            # filler units to interleave into this i's j-loop
            units = []
            if i == 0:
                units.append(lambda: proj_v(0, 0))
                units.append(lambda: proj_v(0, 1))
            if i % 2 == 0:
                units.append(lambda g=i // 2 + 2: dma_x(g))
                for p in range(2):
                    units.append(lambda p=p, b=i // 2 + 1: proj_qk(b, p, "q"))
                for p in range(2):
                    units.append(lambda p=p, b=i // 2 + 1: proj_qk(b, p, "k"))
            units.append(lambda i=i: proj_v(i + 1, 0))
            units.append(lambda i=i: proj_v(i + 1, 1))
            def absorb_one():
                if po_sets:
                    b, c = po_sets.pop(0)
                    for ob in range(8):
                        po_unit(b, ob, c)
            if i >= NI // 2:
                for _ in range(3):
                    units.append(absorb_one)
            if "nofill" in variant:
                for u in units:
                    u()
                units = []

            chunks = [list(range(c0, min(c0 + CH, jmax)))
                      for c0 in range(0, jmax, CH)]
            for cidx, chunk in enumerate(chunks):
                ets = []
                for jn, j in enumerate(chunk):
                    t = j - 2 * i   # 0/1 on the diagonal, <0 off-diagonal
                    stp = stp_pool.tile([128, 1024], F32, name="stp", tag="stp")
                    for p in range(2):
                        for half in range(2):
                            h = 2 * p + half
                            ktx = kta[p] if half == 0 else ktb[p]
                            qtile, qoff = qts[(i, p)]
                            nc.tensor.matmul(
                                stp[:, 256 * h: 256 * (h + 1)],
                                ktx[:, j * 128:(j + 1) * 128],
                                qtile[:, qoff:qoff + 256],
                                start=True, stop=True,
                            )
                    et = etp.tile([128, 1024], BF16, name="et", tag="et")
                    if t == 1:
                        # upper diagonal: only cols 128:256 of each head are
                        # live (qc0 is fully masked and skipped in PV)
                        sl = (lambda tl: tl[:].rearrange(
                            "p (h w) -> p h w", h=4)[:, :, 128:256])
                        nc.scalar.activation(sl(et), sl(stp), EXP, scale=0.125)
                        if "nosel" not in variant:
                            nc.gpsimd.affine_select(
                                out=sl(et), in_=sl(et),
                                compare_op=GE, fill=0.0,
                                base=0, pattern=[[0, 4], [1, 128]],
                                channel_multiplier=-1,
                            )
                    else:
                        nc.scalar.activation(et[:], stp[:], EXP, scale=0.125)
                        if t == 0 and "nosel" not in variant:
                            nc.gpsimd.affine_select(
                                out=et[:].rearrange(
                                    "p (h w) -> p h w", h=4)[:, :, 0:128],
                                in_=et[:].rearrange(
                                    "p (h w) -> p h w", h=4)[:, :, 0:128],
                                compare_op=GE, fill=0.0,
                                base=0, pattern=[[0, 4], [1, 128]],
                                channel_multiplier=-1,
                            )
                    ets.append((et, j))

                    # interleave fillers + previous chunk's PV groups
                    n_emit = -(-len(units) // (jmax - j))
                    for _ in range(n_emit):
                        units.pop(0)()
                    slots_left = len(chunk) - jn
                    n_pv = -(-len(pv_units) // slots_left)
                    for _ in range(n_pv):
                        pv_units.pop(0)()
                ua = (None if "nopv" in variant else
                      ua_pool.tile([128, 520], F32, name="ua", tag="ua"))
                pv_units += pv_units_for(i, cidx, ets, ua, uacc)
                if cidx == len(chunks) - 1:
                    pv_units.append(lambda i=i, u=uacc: norm_unit(i, u))
            for u in units:
                u()

        # tail: drain pending PV/norm + deferred output projections
        for u in pv_units:
            u()
        for b, c in po_sets:
            for ob in range(8):
                po_unit(b, ob, c)
        if dbg:
            for p in range(2):
                nc.sync.dma_start(kt_o.ap()[p * 128:(p + 1) * 128, :], kta[p][:])
            nc.sync.dma_start(vt_o.ap()[:], vt[:])

    nc.compile()
    return nc


def _get_built(T, variant=""):
    key = (T, variant)
    if key not in _BUILD_CACHE:
        _BUILD_CACHE[key] = _build(T, variant)
    return _BUILD_CACHE[key]


def _rearr_w(w):  # [1024, 256] -> [128, 8*256] (d-block major free dim)
    return np.ascontiguousarray(
        w.reshape(8, 128, 256).transpose(1, 0, 2).reshape(128, 8 * 256)
    )


def _numpy_ref(x, mask, Wq, bq, Wk, bk, Wv, bv, Wo, bo):
    T = x.shape[1]
    q = (x @ Wq + bq).reshape(B, T, H, DK).transpose(0, 2, 1, 3)
    k = (x @ Wk + bk).reshape(B, T, H, DK).transpose(0, 2, 1, 3)
    v = (x @ Wv + bv).reshape(B, T, H, DK).transpose(0, 2, 1, 3)
    s = np.einsum("bhqd,bhkd->bhqk", q, k) / np.sqrt(np.float32(DK))
    s = np.where(mask, s, s - 1e9)
    s = s - s.max(axis=-1, keepdims=True)
    e = np.exp(s)
    p = e / e.sum(axis=-1, keepdims=True)
    o = np.einsum("bhqk,bhkd->bhqd", p, v).transpose(0, 2, 1, 3).reshape(B, T, D)
    return (o @ Wo + bo).astype(np.float32)


def kernel(x, mask, Wq, bq, Wk, bk, Wv, bv, Wo, bo):
    from concourse import bass_utils

    x = np.ascontiguousarray(np.asarray(x, dtype=np.float32))
    mask = np.asarray(mask)
    T = x.shape[1]

    causal = bool(
        np.array_equal(mask[0, 0], np.tril(np.ones((T, T), dtype=bool)))
    )
    if not causal or x.shape != (B, T, D) or T % 512 != 0:
        return _numpy_ref(
            np.asarray(x, np.float32), mask,
            np.asarray(Wq, np.float32), np.asarray(bq, np.float32),
            np.asarray(Wk, np.float32), np.asarray(bk, np.float32),
            np.asarray(Wv, np.float32), np.asarray(bv, np.float32),
            np.asarray(Wo, np.float32), np.asarray(bo, np.float32),
        )

    in_maps = _make_in_maps(dict(x=x, Wq=Wq, bq=bq, Wk=Wk, bk=bk,
                                 Wv=Wv, bv=bv, Wo=Wo))
    nc = _get_built(T)
    res = bass_utils.run_bass_kernel_spmd(nc, in_maps, core_ids=list(range(NCORES)))

    out = np.zeros((B, T, D), np.float32)
    for c in range(NCORES):
        out[c // 4] += res.results[c]["ot"].T
    out += np.asarray(bo, np.float32)
    return out


def _make_in_maps(inputs):
    import ml_dtypes
    x = np.ascontiguousarray(np.asarray(inputs["x"], np.float32))
    T = x.shape[1]
    Wq = np.asarray(inputs["Wq"], np.float32)
    Wk = np.asarray(inputs["Wk"], np.float32)
    Wv = np.asarray(inputs["Wv"], np.float32)
    Wo = np.asarray(inputs["Wo"], np.float32)
    bq = np.asarray(inputs["bq"], np.float32)
    bk = np.asarray(inputs["bk"], np.float32)
    bv = np.asarray(inputs["bv"], np.float32)

    xts = [np.ascontiguousarray(x[b].T) for b in range(B)]

    in_maps = []
    for c in range(NCORES):
        b, g = divmod(c, 4)
        cols = slice(g * CW, (g + 1) * CW)
        rows = slice(g * CW, (g + 1) * CW)
        wo_g = Wo[rows].astype(ml_dtypes.bfloat16)  # [256, 1024]
        in_maps.append({
            "xt": xts[b],
            "wq": _rearr_w(Wq[:, cols]),
            "wk": _rearr_w(Wk[:, cols]),
            "wv": _rearr_w(Wv[:, cols]),
            "wo": np.ascontiguousarray(
                wo_g.reshape(2, 128, 1024).transpose(1, 0, 2).reshape(128, 2048)
            ),
            "bqc": np.ascontiguousarray(bq[cols].reshape(2, 128).T),
            "bkc": np.ascontiguousarray(bk[cols].reshape(2, 128).T),
            "bvb": np.ascontiguousarray(
                np.broadcast_to(bv[cols][None, :], (128, 256)).copy()
            ),
            "ident": np.eye(128, dtype=ml_dtypes.bfloat16),
            "zer64": np.zeros((64, T), np.float32),
        })

    return in_maps


# revision 64
# speedup vs baseline: 1.0551x; 1.0227x over previous
"""Multi-head causal attention (B=2, T=4096, D=1024, H=16) on 8 trn2 cores.

Sharding: core c = 4*b + g handles batch b and head-group g (4 heads).
Fully fused single-pass kernel: for each q-block i (256 tokens) the causal
SDPA k-loop runs with exp on ACT as the critical resource; QKV projections
for upcoming blocks, the output projection of the previous block, and DMA
all ride in the PE/DMA slack of the same loop.

Key structure per k-block j (128 keys):
  S^T: 4 matmuls (one per head, 64-dim contraction) -> stp PSUM [128,1024]
  exp: one ACT activation [128,1024] -> et bf16 SBUF
  PV:  et 128x128 chunks as *stationary*, V (64 cols) + ones (1 col) as
       moving -> ctx accumulates as [q,d] in PSUM, softmax denominator Z
       accumulates as a PSUM column -> per-partition normalization.
  ctx.T via PE transpose (bf16, identity matmul) feeds the output
  projection; projections/out-proj ride as PE fillers in the SDPA loop.

Host sums the 4 per-core partial output projections per batch and adds bo.
"""
import numpy as np

B, T0, D, H = 2, 4096, 1024, 16
DK = D // H          # 64
NCORES = 8
HPC = H // 4         # 4 heads per core
CW = HPC * DK        # 256 head-columns per core

_BUILD_CACHE = {}

VG = 260             # vt column group per k-block: 4*64 V + 1 ones + 3 pad


def _build(T, variant=""):
    import concourse.bacc as bacc
    import concourse.mybir as mybir
    import concourse.tile as tile
    from contextlib import ExitStack

    F32 = mybir.dt.float32
    F32R = mybir.dt.float32r
    BF16 = mybir.dt.bfloat16
    EXP = mybir.ActivationFunctionType.Exp
    GE = mybir.AluOpType.is_ge

    NI = T // 256    # q-blocks of 256
    NJ = T // 128    # k-blocks of 128

    nc = bacc.Bacc("TRN2", target_bir_lowering=False, debug=False, num_devices=8)

    xt_d = nc.dram_tensor("xt", [D, T], F32R, kind="ExternalInput")
    wq_d = nc.dram_tensor("wq", [128, 8 * 256], F32R, kind="ExternalInput")
    wk_d = nc.dram_tensor("wk", [128, 8 * 256], F32R, kind="ExternalInput")
    wv_d = nc.dram_tensor("wv", [128, 8 * 256], F32R, kind="ExternalInput")
    wo_d = nc.dram_tensor("wo", [128, 2 * 1024], BF16, kind="ExternalInput")
    bqc_d = nc.dram_tensor("bqc", [128, 2], F32, kind="ExternalInput")
    bkc_d = nc.dram_tensor("bkc", [128, 2], F32, kind="ExternalInput")
    bvb_d = nc.dram_tensor("bvb", [128, 256], F32, kind="ExternalInput")
    idn_d = nc.dram_tensor("ident", [128, 128], BF16, kind="ExternalInput")
    zer_d = nc.dram_tensor("zer64", [64, T], F32R, kind="ExternalInput")
    ot_d = nc.dram_tensor("ot", [D, T], F32, kind="ExternalOutput")
    dbg = "dbg" in variant
    if dbg or "dA" in variant or "dB" in variant:
        NJ_ = T // 128
        qt_o = nc.dram_tensor("qt_o", [2 * 128, T], F32R, kind="ExternalOutput")
        kt_o = nc.dram_tensor("kt_o", [2 * 128, T], F32R, kind="ExternalOutput")
        vt_o = nc.dram_tensor("vt_o", [128, NJ_ * VG], BF16, kind="ExternalOutput")
        ua_o = nc.dram_tensor("ua_o", [T // 256 * 128, 512], F32, kind="ExternalOutput")
        zz_o = nc.dram_tensor("zz_o", [T // 256 * 128, 8], F32, kind="ExternalOutput")
        cx_o = nc.dram_tensor("cx_o", [T // 256 * 128, 512], BF16, kind="ExternalOutput")

    with tile.TileContext(nc) as tc, ExitStack() as ctx:
        ctx.enter_context(nc.allow_low_precision(reason="fp32r/bf16 by design"))

        # ---- persistent SBUF ----
        # kta[p]: rows 0:64 = K^T of head 2p, rows 64:128 zero.
        # ktb[p]: rows 64:128 = K^T of head 2p+1, rows 0:64 zero.
        # (Full-128-partition S matmuls; 64-partition operands crash HW.)
        per = ctx.enter_context(tc.tile_pool(name="persist", bufs=1))
        kta = [per.tile([128, T], F32R, name=f"kta{p}", tag=f"kta{p}")
               for p in range(2)]
        ktb = [per.tile([128, T], F32R, name=f"ktb{p}", tag=f"ktb{p}")
               for p in range(2)]
        vt = per.tile([128, NJ * VG], BF16, name="vt", tag="vt")
        wq_sb = per.tile([128, 2048], F32R, name="wq", tag="wq")
        wk_sb = per.tile([128, 2048], F32R, name="wk", tag="wk")
        wv_sb = per.tile([128, 2048], F32R, name="wv", tag="wv")
        wo_sb = per.tile([128, 2, 1024], BF16, name="wo", tag="wo")
        bqc_sb = per.tile([128, 2], F32, name="bqc", tag="bqc")
        bkc_sb = per.tile([128, 2], F32, name="bkc", tag="bkc")
        bvb_sb = per.tile([128, 256], F32, name="bvb", tag="bvb")
        idn_sb = per.tile([128, 128], BF16, name="idn", tag="idn")

        # ones column per k-group (for Z matmuls); pad cols stay whatever.
        vt4 = vt[:].rearrange("p (j c) -> p j c", c=VG)[:, :, 256:VG]
        nc.vector.memset(vt4, 1.0)

        # ---- pools ----
        xpool = ctx.enter_context(tc.tile_pool(name="xts", bufs=16))
        etp = ctx.enter_context(tc.tile_pool(name="etp", bufs=9))
        qtp = ctx.enter_context(tc.tile_pool(name="qtp", bufs=3))
        cxp = ctx.enter_context(tc.tile_pool(name="cxp", bufs=2))
        cxtp = ctx.enter_context(tc.tile_pool(name="cxtp", bufs=20))
        otp = ctx.enter_context(tc.tile_pool(name="otp", bufs=1))
        uap = ctx.enter_context(tc.tile_pool(name="uap", bufs=2))

        stp_pool = ctx.enter_context(tc.tile_pool(name="stps", bufs=2, space="PSUM"))
        ua_pool = ctx.enter_context(tc.tile_pool(name="uaps", bufs=1, space="PSUM"))
        pp_pool = ctx.enter_context(tc.tile_pool(name="ppps", bufs=2, space="PSUM"))
        po_pool = pp_pool  # shared ring: proj and output-proj tiles rotate together

        xts = {}          # gen -> list of 8 tiles [128, 512]

        def dma_x(gen, half=None):
            if gen * 512 >= T:
                return
            if gen not in xts:
                xts[gen] = [xpool.tile([128, 512], F32R, name="xt", tag="xt")
                            for _ in range(8)]
            lo, hi = {None: (0, 512), 0: (0, 256), 1: (256, 512)}[half]
            for db, xtile in enumerate(xts[gen]):
                nc.sync.dma_start(
                    xtile[:, lo:hi],
                    xt_d.ap()[db * 128:(db + 1) * 128,
                              gen * 512 + lo:gen * 512 + hi],
                )

        qts = {}          # (i, p) -> (q^T tile [128, 512], col offset)

        def proj_qk(bp, p, which):
            """Project q or k for block-pair bp (blocks 2bp, 2bp+1), pair p."""
            if 2 * bp >= NI:
                return
            w_sb, bias = (wq_sb, bqc_sb) if which == "q" else (wk_sb, bkc_sb)
            ps = pp_pool.tile([128, 512], F32, name="pp", tag="pp")
            for db in range(8):
                nc.tensor.matmul(
                    ps[:],
                    w_sb[:, db * 256 + p * 128: db * 256 + (p + 1) * 128],
                    xts[bp][db][:],
                    start=(db == 0), stop=(db == 7),
                )
            if which == "q":
                qtile = qtp.tile([128, 512], F32R, name="qt", tag=f"qt{p}")
                nc.vector.tensor_scalar_add(qtile[:], ps[:], bias[:, p:p + 1])
                qts[(2 * bp, p)] = (qtile, 0)
                qts[(2 * bp + 1, p)] = (qtile, 256)
            else:
                nc.vector.tensor_scalar_add(
                    kta[p][0:64, bp * 512:(bp + 1) * 512], ps[0:64, :],
                    bias[0:64, p:p + 1]
                )
                nc.vector.tensor_scalar_add(
                    ktb[p][64:128, bp * 512:(bp + 1) * 512], ps[64:128, :],
                    bias[64:128, p:p + 1]
                )

        def proj_v(i, jb):
            """Project v for k-block j = 2*i + jb -> vt group."""
            if i >= NI:
                return
            gen, off = divmod(i, 2)
            j = 2 * i + jb
            ps = pp_pool.tile([128, 256], F32, name="pp", tag="pp")
            tok = off * 256 + jb * 128
            for db in range(8):
                nc.tensor.matmul(
                    ps[:],
                    xts[gen][db][:, tok:tok + 128],
                    wv_sb[:, db * 256:(db + 1) * 256],
                    start=(db == 0), stop=(db == 7),
                )
            nc.vector.tensor_add(
                vt[:, j * VG: j * VG + 256], ps[:], bvb_sb[:]
            )

        ot_stage = {}     # ob -> staging tile [128, 512] across an i-pair

        def po_unit(i, ob, cxT):
            """Output projection for q-block i, output row block ob."""
            po = po_pool.tile([128, 256], F32, name="po", tag="pp")
            for qc in range(2):
                for h in range(2):
                    if "nopo" in variant:
                        continue
                    nc.tensor.matmul(
                        po[:, qc * 128:(qc + 1) * 128],
                        wo_sb[:, h, ob * 128:(ob + 1) * 128],
                        cxT[qc][:, h, :],
                        start=(h == 0), stop=(h == 1),
                    )
            if "nopo" in variant:
                nc.vector.memset(po[:], 0.0)
            last_pair = (i // 2 == NI // 2 - 1)
            if i % 2 == 0:
                stage = otp.tile([128, 512], F32, name="ots", tag=f"ots{ob}")
                ot_stage[ob] = stage
                nc.vector.tensor_copy(stage[:, 0:256], po[:])
                if last_pair:
                    nc.sync.dma_start(
                        ot_d.ap()[ob * 128:(ob + 1) * 128,
                                  i * 256:(i + 1) * 256],
                        stage[:, 0:256],
                    )
            else:
                stage = ot_stage[ob]
                nc.vector.tensor_copy(stage[:, 256:512], po[:])
                if last_pair:
                    nc.sync.dma_start(
                        ot_d.ap()[ob * 128:(ob + 1) * 128,
                                  i * 256:(i + 1) * 256],
                        stage[:, 256:512],
                    )
                else:
                    nc.sync.dma_start(
                        ot_d.ap()[ob * 128:(ob + 1) * 128,
                                  (i - 1) * 256:(i + 1) * 256],
                        stage[:],
                    )

        po_sets = []      # deferred output-projection work: (block, cxT)
        CH = 4            # k-blocks per PSUM accumulation chunk

        def norm_unit(i, uacc):
            """Normalize block i's ctx, transpose it, queue its out-proj."""
            if dbg or "dA" in variant:
                nc.sync.dma_start(ua_o.ap()[i * 128:(i + 1) * 128, :],
                                  uacc[:, 0:512])
                nc.sync.dma_start(zz_o.ap()[i * 128:(i + 1) * 128, :],
                                  uacc[:, 512:520])
            rz = cxp.tile([128, 8], F32, name="rz", tag="rz")
            nc.vector.reciprocal_approx_fast(out=rz[:], in_=uacc[:, 512:520])
            cxT = []
            for qc in range(2):
                cx = cxp.tile([128, 256], BF16, name="cx", tag="cx")
                for h in range(4):
                    nc.vector.tensor_scalar_mul(
                        cx[:, 64 * h: 64 * (h + 1)],
                        uacc[:, 256 * qc + 64 * h: 256 * qc + 64 * (h + 1)],
                        rz[:, 4 * qc + h: 4 * qc + h + 1],
                    )
                ct = cxtp.tile([128, 2, 128], BF16, name="cxT", tag="cxT")
                if "xbar" in variant:
                    nc.sync.dma_start_transpose(ct[:], cx[:])
                else:
                    tps = po_pool.tile([128, 256], BF16, name="tp", tag="pp")
                    for hh in range(2):
                        nc.tensor.transpose(
                            tps[:, 128 * hh:128 * (hh + 1)],
                            cx[:, 128 * hh:128 * (hh + 1)],
                            idn_sb[:],
                        )
                    nc.vector.tensor_copy(
                        ct[:].rearrange("p a b -> p (a b)"), tps[:]
                    )
                if dbg or "dB" in variant:
                    nc.sync.dma_start(
                        cx_o.ap()[i * 128:(i + 1) * 128,
                                  qc * 256:(qc + 1) * 256], cx[:]
                    )
                cxT.append(ct)
            po_sets.append((i, cxT))

        def pv_units_for(i, cidx, ets, ua, uacc):
            """Closed PV+Z accumulation group closures for one chunk + drain.

            PV groups live in bank 0 of `ua` (cols 0:512), Z groups in
            bank 1 (cols 512:520); groups are sequential per bank so each
            start/stop pair owns its zero-region exclusively.
            """
            def group(qc, h):
                if "nopv" in variant:
                    return
                js = [(et, j) for (et, j) in ets
                      if not (j == 2 * i + 1 and qc == 0)]
                for idx, (et, j) in enumerate(js):
                    flags = dict(
                        start=(idx == 0), stop=(idx == len(js) - 1),
                        skip_group_check=True,
                    )
                    lhs = et[:, 256 * h + 128 * qc: 256 * h + 128 * (qc + 1)]
                    nc.tensor.matmul(
                        ua[:, 256 * qc + 64 * h: 256 * qc + 64 * (h + 1)],
                        lhs,
                        vt[:, j * VG + 64 * h: j * VG + 64 * (h + 1)],
                        **flags,
                    )
                    if "nozc" not in variant:
                        nc.tensor.matmul(
                            ua[:, 512 + 4 * qc + h: 512 + 4 * qc + h + 1],
                            lhs,
                            vt[:, j * VG + 256: j * VG + 257],
                            **flags,
                        )
                    elif h == 0:
                        nc.tensor.matmul(
                            ua[:, 512 + 4 * qc: 512 + 4 * qc + 4],
                            lhs,
                            vt[:, j * VG + 64: j * VG + 68],
                            **flags,
                        )

            def drain():
                if "nopv" in variant:
                    if cidx == 0:
                        nc.vector.memset(uacc[:], 1.0)
                    return
                if cidx == 0:
                    nc.vector.tensor_copy(uacc[:], ua[:])
                else:
                    nc.vector.tensor_add(uacc[:], uacc[:], ua[:])

            out = [lambda qc=qc, h=h: group(qc, h)
                   for qc in range(2) for h in range(4)]
            out.append(drain)
            return out

        # ---- bootstrap (DMA order matters: x halves + qk weights first) ----
        dma_x(0, half=0)
        nc.sync.dma_start(wq_sb[:], wq_d.ap()[:])
        nc.sync.dma_start(bqc_sb[:], bqc_d.ap()[:])
        dma_x(0, half=1)
        nc.sync.dma_start(wk_sb[:], wk_d.ap()[:])
        nc.sync.dma_start(bkc_sb[:], bkc_d.ap()[:])
        for p in range(2):
            nc.sync.dma_start(kta[p][64:128, :], zer_d.ap()[:])
            nc.sync.dma_start(ktb[p][0:64, :], zer_d.ap()[:])
        for p in range(2):
            proj_qk(0, p, "q")
        nc.sync.dma_start(wv_sb[:], wv_d.ap()[:])
        nc.sync.dma_start(bvb_sb[:], bvb_d.ap()[:])
        for p in range(2):
            proj_qk(0, p, "k")
        nc.sync.dma_start(wo_sb[:].rearrange("p a b -> p (a b)"), wo_d.ap()[:])
        nc.sync.dma_start(idn_sb[:], idn_d.ap()[:])
        dma_x(1)

        pv_units = []   # pending PV/drain/norm closures, carried across blocks
        for i in range(NI):
            jmax = 2 * i + 2
            uacc = uap.tile([128, 520], F32, name="uacc", tag="uacc")

            # filler units to interleave into this i's j-loop
            units = []
            if i == 0:
                units.append(lambda: proj_v(0, 0))
                units.append(lambda: proj_v(0, 1))
            if i % 2 == 0:
                units.append(lambda g=i // 2 + 2: dma_x(g))
                for p in range(2):
                    units.append(lambda p=p, b=i // 2 + 1: proj_qk(b, p, "q"))
            else:
                # K of the next pair is first read one block later than Q,
                # so its projection rides in the lighter odd blocks.
                for p in range(2):
                    units.append(
                        lambda p=p, b=(i + 1) // 2: proj_qk(b, p, "k"))
            units.append(lambda i=i: proj_v(i + 1, 0))
            units.append(lambda i=i: proj_v(i + 1, 1))
            def absorb_one():
                if po_sets:
                    b, c = po_sets.pop(0)
                    for ob in range(8):
                        po_unit(b, ob, c)
            if i >= NI // 2:
                for _ in range(3):
                    units.append(absorb_one)
            if "nofill" in variant:
                for u in units:
                    u()
                units = []

            chunks = [list(range(c0, min(c0 + CH, jmax)))
                      for c0 in range(0, jmax, CH)]
            for cidx, chunk in enumerate(chunks):
                ets = []
                for jn, j in enumerate(chunk):
                    t = j - 2 * i   # 0/1 on the diagonal, <0 off-diagonal
                    stp = stp_pool.tile([128, 1024], F32, name="stp", tag="stp")
                    for p in range(2):
                        for half in range(2):
                            h = 2 * p + half
                            ktx = kta[p] if half == 0 else ktb[p]
                            qtile, qoff = qts[(i, p)]
                            nc.tensor.matmul(
                                stp[:, 256 * h: 256 * (h + 1)],
                                ktx[:, j * 128:(j + 1) * 128],
                                qtile[:, qoff:qoff + 256],
                                start=True, stop=True,
                            )
                    et = etp.tile([128, 1024], BF16, name="et", tag="et")
                    if t == 1:
                        # upper diagonal: only cols 128:256 of each head are
                        # live (qc0 is fully masked and skipped in PV)
                        sl = (lambda tl: tl[:].rearrange(
                            "p (h w) -> p h w", h=4)[:, :, 128:256])
                        nc.scalar.activation(sl(et), sl(stp), EXP, scale=0.125)
                        if "nosel" not in variant:
                            nc.gpsimd.affine_select(
                                out=sl(et), in_=sl(et),
                                compare_op=GE, fill=0.0,
                                base=0, pattern=[[0, 4], [1, 128]],
                                channel_multiplier=-1,
                            )
                    else:
                        nc.scalar.activation(et[:], stp[:], EXP, scale=0.125)
                        if t == 0 and "nosel" not in variant:
                            nc.gpsimd.affine_select(
                                out=et[:].rearrange(
                                    "p (h w) -> p h w", h=4)[:, :, 0:128],
                                in_=et[:].rearrange(
                                    "p (h w) -> p h w", h=4)[:, :, 0:128],
                                compare_op=GE, fill=0.0,
                                base=0, pattern=[[0, 4], [1, 128]],
                                channel_multiplier=-1,
                            )
                    ets.append((et, j))

                    # interleave fillers + previous chunk's PV groups
                    n_emit = -(-len(units) // (jmax - j))
                    for _ in range(n_emit):
                        units.pop(0)()
                    slots_left = len(chunk) - jn
                    n_pv = -(-len(pv_units) // slots_left)
                    for _ in range(n_pv):
                        pv_units.pop(0)()
                ua = (None if "nopv" in variant else
                      ua_pool.tile([128, 520], F32, name="ua", tag="ua"))
                pv_units += pv_units_for(i, cidx, ets, ua, uacc)
                if cidx == len(chunks) - 1:
                    pv_units.append(lambda i=i, u=uacc: norm_unit(i, u))
            for u in units:
                u()

        # tail: drain pending PV/norm + deferred output projections
        for u in pv_units:
            u()
        for b, c in po_sets:
            for ob in range(8):
                po_unit(b, ob, c)
        if dbg:
            for p in range(2):
                nc.sync.dma_start(kt_o.ap()[p * 128:(p + 1) * 128, :], kta[p][:])
            nc.sync.dma_start(vt_o.ap()[:], vt[:])

    nc.compile()
    return nc


def _get_built(T, variant=""):
    key = (T, variant)
    if key not in _BUILD_CACHE:
        _BUILD_CACHE[key] = _build(T, variant)
    return _BUILD_CACHE[key]


def _rearr_w(w):  # [1024, 256] -> [128, 8*256] (d-block major free dim)
    return np.ascontiguousarray(
        w.reshape(8, 128, 256).transpose(1, 0, 2).reshape(128, 8 * 256)
    )


def _numpy_ref(x, mask, Wq, bq, Wk, bk, Wv, bv, Wo, bo):
    T = x.shape[1]
    q = (x @ Wq + bq).reshape(B, T, H, DK).transpose(0, 2, 1, 3)
    k = (x @ Wk + bk).reshape(B, T, H, DK).transpose(0, 2, 1, 3)
    v = (x @ Wv + bv).reshape(B, T, H, DK).transpose(0, 2, 1, 3)
    s = np.einsum("bhqd,bhkd->bhqk", q, k) / np.sqrt(np.float32(DK))
    s = np.where(mask, s, s - 1e9)
    s = s - s.max(axis=-1, keepdims=True)
    e = np.exp(s)
    p = e / e.sum(axis=-1, keepdims=True)
    o = np.einsum("bhqk,bhkd->bhqd", p, v).transpose(0, 2, 1, 3).reshape(B, T, D)
    return (o @ Wo + bo).astype(np.float32)


def kernel(x, mask, Wq, bq, Wk, bk, Wv, bv, Wo, bo):
    from concourse import bass_utils

    x = np.ascontiguousarray(np.asarray(x, dtype=np.float32))
    mask = np.asarray(mask)
    T = x.shape[1]

    causal = bool(
        np.array_equal(mask[0, 0], np.tril(np.ones((T, T), dtype=bool)))
    )
    if not causal or x.shape != (B, T, D) or T % 512 != 0:
        return _numpy_ref(
            np.asarray(x, np.float32), mask,
            np.asarray(Wq, np.float32), np.asarray(bq, np.float32),
            np.asarray(Wk, np.float32), np.asarray(bk, np.float32),
            np.asarray(Wv, np.float32), np.asarray(bv, np.float32),
            np.asarray(Wo, np.float32), np.asarray(bo, np.float32),
        )

    in_maps = _make_in_maps(dict(x=x, Wq=Wq, bq=bq, Wk=Wk, bk=bk,
                                 Wv=Wv, bv=bv, Wo=Wo))
    nc = _get_built(T)
    res = bass_utils.run_bass_kernel_spmd(nc, in_maps, core_ids=list(range(NCORES)))

    out = np.zeros((B, T, D), np.float32)
    for c in range(NCORES):
        out[c // 4] += res.results[c]["ot"].T
    out += np.asarray(bo, np.float32)
    return out


def _make_in_maps(inputs):
    import ml_dtypes
    x = np.ascontiguousarray(np.asarray(inputs["x"], np.float32))
    T = x.shape[1]
    Wq = np.asarray(inputs["Wq"], np.float32)
    Wk = np.asarray(inputs["Wk"], np.float32)
    Wv = np.asarray(inputs["Wv"], np.float32)
    Wo = np.asarray(inputs["Wo"], np.float32)
    bq = np.asarray(inputs["bq"], np.float32)
    bk = np.asarray(inputs["bk"], np.float32)
    bv = np.asarray(inputs["bv"], np.float32)

    xts = [np.ascontiguousarray(x[b].T) for b in range(B)]

    in_maps = []
    for c in range(NCORES):
        b, g = divmod(c, 4)
        cols = slice(g * CW, (g + 1) * CW)
        rows = slice(g * CW, (g + 1) * CW)
        wo_g = Wo[rows].astype(ml_dtypes.bfloat16)  # [256, 1024]
        in_maps.append({
            "xt": xts[b],
            "wq": _rearr_w(Wq[:, cols]),
            "wk": _rearr_w(Wk[:, cols]),
            "wv": _rearr_w(Wv[:, cols]),
            "wo": np.ascontiguousarray(
                wo_g.reshape(2, 128, 1024).transpose(1, 0, 2).reshape(128, 2048)
            ),
            "bqc": np.ascontiguousarray(bq[cols].reshape(2, 128).T),
            "bkc": np.ascontiguousarray(bk[cols].reshape(2, 128).T),
            "bvb": np.ascontiguousarray(
                np.broadcast_to(bv[cols][None, :], (128, 256)).copy()
            ),
            "ident": np.eye(128, dtype=ml_dtypes.bfloat16),
            "zer64": np.zeros((64, T), np.float32),
        })

    return in_maps
